# revision 29
# baseline (speedup 1.0000x reference)
"""Trainium2 Bass kernel for nn_KAN_63230508532179 (dense_mlp).

Model (per reference):
  h = gelu(x[:,:,None] * bw1 + bb1)            # [B,1000,16]
  f = tanh(einsum('bnh,noh->bno', h, bw2)+bb2) # [B,1000,8]
  z = f.reshape(B, 8000)
  z = gelu(z @ wc1.T + bc1)                    # [B,256]
  z = gelu(z @ wc2.T + bc2)                    # [B,128]
  y = z @ wc3.T + bc3                          # [B,300]

Strategy: data-parallel over batch across 8 cores (512 rows each); all
on-chip tensors transposed ([feature, batch]).  Branches run in
8-branch groups, two groups per combiner-1 K-chunk.  Layer 1 is K=27
row-tiled matmuls: x ships in supertiles of 12 groups (4 strips x 3
group-slots of 9 rows: 8 x-rows + a ones row, so scale and bias fold
into the weights) and each group's lhsT is a [27,128] block that is
zero outside its 9 rows.  Layer 2 is K=128 M=64 matmuls with 2-way
column concurrency; its bias rides the tanh activation's per-partition
bias port.  Combiner-1 accumulates 63 K-chunks into persistent PSUM.
Every constant is split into per-chunk tiles so Tile's dependency
tracking lets compute start as soon as the first slices land, and the
ACT stream is software-pipelined (tanh of chunk c-1 issues after gelu
of chunk c) so the scalar engine never waits on layer-2 latency.
"""

import os
import sys
from contextlib import ExitStack

sys.path.insert(0, "/opt/trn_rl_repo")
os.environ.setdefault("MYCRO_LOCAL_CACHE", "1")

import numpy as np
import ml_dtypes

import concourse.bass as bass
import concourse.tile as tile
from concourse import bacc, mybir
from concourse.bass_utils import run_bass_kernel_spmd

BF16 = mybir.dt.bfloat16
F32 = mybir.dt.float32
NPBF16 = ml_dtypes.bfloat16

B, N, H1, H2 = 4096, 1000, 16, 8
C1, C2, OUT = 256, 128, 300
NCORES = 8
BC = B // NCORES          # 512 batch rows per core
NCH = 63                  # combiner-1 K-chunks (2 groups each)
NG = 128                  # padded groups of 8 branches (125 real)
NP_ = NG * 8              # 1024 padded branches
NST = 11                  # x supertiles of 12 groups (132 slots)

_CACHE = {}


def _gloc(g):
    """group -> (supertile, strip, slot)"""
    return g // 12, (g % 12) // 3, g % 3


def _build_program():
    if "nc" in _CACHE:
        return _CACHE["nc"]

    nc = bacc.Bacc("TRN2", target_bir_lowering=False, debug=False,
                   num_devices=NCORES)

    xq_d = nc.dram_tensor("xq", [NST * 128, BC], BF16, kind="ExternalInput")
    w1_d = nc.dram_tensor("w1", [NST * 128, 384], BF16, kind="ExternalInput")
    w2_d = nc.dram_tensor("w2", [128, NG * 64], BF16, kind="ExternalInput")
    # f32 smalls packed: b2 (64) | bc1 (2) | bc2 (1) | bc3 (3)
    sf_d = nc.dram_tensor("sf", [128, 70], F32, kind="ExternalInput")
    wc1_d = nc.dram_tensor("wc1", [128, 64 * 256], BF16, kind="ExternalInput")
    # bf16 smalls packed: wc2 (256) | wc3 (300)
    sb_d = nc.dram_tensor("sb", [128, 556], BF16, kind="ExternalInput")
    out_d = nc.dram_tensor("out", [128, 3 * BC], F32, kind="ExternalOutput")

    AF = mybir.ActivationFunctionType

    with ExitStack() as ctx:
        tc = ctx.enter_context(tile.TileContext(nc))
        consts = ctx.enter_context(tc.tile_pool(name="consts", bufs=1))
        h_pool = ctx.enter_context(tc.tile_pool(name="h", bufs=4))
        f_pool = ctx.enter_context(tc.tile_pool(name="f", bufs=4))
        z_pool = ctx.enter_context(tc.tile_pool(name="z", bufs=1))
        ps_h = ctx.enter_context(tc.tile_pool(name="psh", bufs=2, space="PSUM"))
        ps_f = ctx.enter_context(tc.tile_pool(name="psf", bufs=2, space="PSUM"))
        ps_z = ctx.enter_context(tc.tile_pool(name="psz", bufs=1, space="PSUM"))

        # ---- PE warm-up tile: matmuls on it accumulate into z1a, which the
        # first real combiner-1 matmul (start=True) clears, so the values
        # never matter -- they only give the HAM clock gate the gap-free
        # busy window it needs to reach 8/8 (per-chunk sem micro-gaps never
        # do).  Left uninitialized so the burst needs no memset and starts
        # the moment the engines boot.
        warm_sb = consts.tile([128, BC], BF16, tag="warm")

        # ---- chunked constants: one tile per slice => fine-grained deps.
        # Sync issues DMA descriptors serially (~0.6us each), so order by
        # first use: chunk 0's tiles, then f32 smalls, then the rest.
        xe, w1t, w2t, wc1t = [], [], [], []
        sfs = consts.tile([128, 70], F32, tag="sf")
        sbs = consts.tile([128, 556], BF16, tag="sbc")
        for v in range(NST):
            xt = consts.tile([128, BC], BF16, tag=f"xe{v}")
            nc.sync.dma_start(out=xt[:], in_=xq_d[128 * v:128 * (v + 1), :])
            xe.append(xt)
            wt = consts.tile([128, 384], BF16, tag=f"w1_{v}")
            nc.sync.dma_start(out=wt[:], in_=w1_d[128 * v:128 * (v + 1), :])
            w1t.append(wt)
            if v < 8:
                w2c = consts.tile([128, 1024], BF16, tag=f"w2_{v}")
                nc.sync.dma_start(out=w2c[:],
                                  in_=w2_d[:, 1024 * v:1024 * (v + 1)])
                w2t.append(w2c)
                wcc = consts.tile([128, 2048], BF16, tag=f"wc1_{v}")
                nc.sync.dma_start(out=wcc[:],
                                  in_=wc1_d[:, 2048 * v:2048 * (v + 1)])
                wc1t.append(wcc)
            if v == 0:
                nc.sync.dma_start(out=sfs[:], in_=sf_d[:, :])
            elif v == 1:
                nc.sync.dma_start(out=sbs[:], in_=sb_d[:, :])
        b2s = sfs[:, 0:64]
        bc1s = sfs[:, 64:66]
        bc2s = sfs[:, 66:67]
        bc3s = sfs[:, 67:70]
        wc2s = sbs[:, 0:256]
        wc3s = sbs[:, 256:556]

        def w1_ap_g(g):
            t, i, u = _gloc(g)
            return w1t[t][32 * i:32 * i + 27, 128 * u:128 * (u + 1)]

        def x_ap_g(g):
            t, i, u = _gloc(g)
            return xe[t][32 * i:32 * i + 27, :]

        def w2_ap_g(g):
            return w2t[g // 16][:, 64 * (g % 16):64 * (g % 16 + 1)]

        def wc1_ap(c, half):
            return wc1t[c // 8][:, 256 * (c % 8) + 128 * half:
                                256 * (c % 8) + 128 * (half + 1)]

        # ---- main loop: 63 chunks, ACT software-pipelined by one chunk ----
        z1a_ps = ps_z.tile([128, BC], F32, tag="z1a")
        z1b_ps = ps_z.tile([128, BC], F32, tag="z1b")
        fps_q = [None, None]

        def warm_burst(n, start):
            for k in range(n):
                nc.tensor.matmul(z1a_ps[:], lhsT=warm_sb[:, 0:128],
                                 rhs=warm_sb[:], start=(start and k == 0),
                                 stop=False, skip_group_check=True)

        warm_burst(10, True)   # garbage data; z1a cleared by chunk 0's C1
        # zero warm_sb so the mid-loop re-warm bursts (which land after the
        # real accumulation begins) add exact zeros to z1a
        nc.vector.memset(warm_sb[:], 0.0)

        def emit_tanh(c):
            fsb = f_pool.tile([128, BC], BF16, tag="fsb")
            nc.scalar.activation(fsb[:], fps_q[c % 2][:], AF.Tanh,
                                 bias=b2s[:, c:c + 1], scale=1.0)
            return fsb

        def emit_c1(c, fsb):
            last = c == NCH - 1
            nc.tensor.matmul(z1a_ps[:], lhsT=wc1_ap(c, 0), rhs=fsb[:],
                             start=(c == 0), stop=last, skip_group_check=True)
            nc.tensor.matmul(z1b_ps[:], lhsT=wc1_ap(c, 1), rhs=fsb[:],
                             start=(c == 0), stop=last, skip_group_check=True)

        for c in range(NCH):
            hps = ps_h.tile([128, 2 * BC], F32, tag="hps")
            for j in range(2):
                g = 2 * c + j
                i = (g % 12) // 3
                nc.tensor.matmul(
                    hps[:, BC * j:BC * (j + 1)],
                    lhsT=w1_ap_g(g), rhs=x_ap_g(g),
                    start=True, stop=True, tile_position=(32 * i, 0))
            hsb = h_pool.tile([128, 2 * BC], BF16, tag="hsb")
            nc.scalar.activation(hsb[:], hps[:], AF.Gelu)
            fps = ps_f.tile([128, BC], F32, tag="fps")
            fps_q[c % 2] = fps
            for j in range(2):
                g = 2 * c + j
                nc.tensor.matmul(
                    fps[64 * j:64 * (j + 1), :],
                    lhsT=w2_ap_g(g),
                    rhs=hsb[:, BC * j:BC * (j + 1)],
                    start=True, stop=True, tile_position=(0, 64 * j))
            fsb = emit_tanh(c)
            emit_c1(c, fsb)
            if c in (1, 3, 5):
                # keep the HAM busy-window alive through pipeline fill
                warm_burst(8, False)

        # ---- combiner tail ----
        z1a = z_pool.tile([128, BC], BF16, tag="z1a_sb")
        z1b = z_pool.tile([128, BC], BF16, tag="z1b_sb")
        z2_ps = ps_h.tile([128, BC], F32, tag="hps")
        nc.scalar.activation(z1a[:], z1a_ps[:], AF.Gelu,
                             bias=bc1s[:, 0:1], scale=1.0)
        nc.tensor.matmul(z2_ps[:], lhsT=wc2s[:, 0:128], rhs=z1a[:],
                         start=True, stop=False, skip_group_check=True)
        nc.scalar.activation(z1b[:], z1b_ps[:], AF.Gelu,
                             bias=bc1s[:, 1:2], scale=1.0)
        nc.tensor.matmul(z2_ps[:], lhsT=wc2s[:, 128:256], rhs=z1b[:],
                         start=False, stop=True, skip_group_check=True)
        z2 = z_pool.tile([128, BC], BF16, tag="z2_sb")
        nc.scalar.activation(z2[:], z2_ps[:], AF.Gelu,
                             bias=bc2s[:, 0:1], scale=1.0)

        o_sb = z_pool.tile([128, 3 * BC], F32, tag="osb")
        for i, m in ((0, 128), (1, 128), (2, 44)):
            o_ps = ps_f.tile([128, BC], F32, tag="fps")
            nc.tensor.matmul(o_ps[0:m, :], lhsT=wc3s[:, 128 * i:128 * i + m],
                             rhs=z2[:], start=True, stop=True)
            nc.vector.tensor_scalar_add(o_sb[0:m, BC * i:BC * (i + 1)],
                                        o_ps[0:m, :], bc3s[0:m, i:i + 1])
            if i == 1:
                nc.sync.dma_start(out=out_d[:, 0:2 * BC],
                                  in_=o_sb[:, 0:2 * BC])
        nc.sync.dma_start(out=out_d[:, 2 * BC:], in_=o_sb[:, 2 * BC:])

    nc.compile()
    _CACHE["nc"] = nc
    return nc


def preprocess(x, bw1, bb1, bw2, bb2, wc1, bc1, wc2, bc2, wc3, bc3):
    """Host-side repack of full inputs into per-core input maps."""
    f32 = np.float32
    bw1p = np.zeros((NST * 12 * 8, H1), f32); bw1p[:N] = bw1
    bb1p = np.zeros((NST * 12 * 8, H1), f32); bb1p[:N] = bb1
    bw2p = np.zeros((NP_, H2, H1), f32); bw2p[:N] = bw2
    bb2p = np.zeros((NP_, H2), f32); bb2p[:N] = bb2

    # x supertiles: [t, 128, B]; strip i, slot u, row r (8 x rows + ones)
    xr = np.zeros((NST * 12 * 8, B), f32)
    xr[:N] = x.T
    xrg = xr.reshape(NST, 4, 3, 8, B)          # [t, i, u, r, b]
    xq = np.zeros((NST, 4, 32, B), f32)
    for u in range(3):
        xq[:, :, 9 * u:9 * u + 8, :] = xrg[:, :, u]
        xq[:, :, 9 * u + 8, :] = 1.0
    xq = xq.reshape(NST * 128, B).astype(NPBF16)

    # w1 blocks: [t, i (32-row strip), z=9u+r, 128u + (16 bb + k)]
    W1 = np.zeros((NST, 4, 32, 384), f32)
    bw1g = bw1p.reshape(NST, 4, 3, 8, H1)      # [t, i, u, bb, k]
    bb1g = bb1p.reshape(NST, 4, 3, 8, H1)
    for u in range(3):
        for bb in range(8):
            W1[:, :, 9 * u + bb, 128 * u + 16 * bb:128 * u + 16 * bb + 16] = \
                bw1g[:, :, u, bb]
            W1[:, :, 9 * u + 8, 128 * u + 16 * bb:128 * u + 16 * bb + 16] = \
                bb1g[:, :, u, bb]
    w1_sb = W1.reshape(NST * 128, 384).astype(NPBF16)

    # w2 block-diagonal per group: [128=(bb,k), 64=(bb,o)]
    W2 = np.zeros((NG, 128, 64), f32)
    bw2g = bw2p.reshape(NG, 8, H2, H1)         # [g, bb, o, k]
    for bb in range(8):
        W2[:, 16 * bb:16 * (bb + 1), 8 * bb:8 * (bb + 1)] = \
            bw2g[:, bb].transpose(0, 2, 1)     # [g, k, o]
    w2_sb = W2.transpose(1, 0, 2).reshape(128, NG * 64).astype(NPBF16)
    b2_sb = bb2p.reshape(64, 128).T

    # combiner 1: wc1 [256, 8000] -> chunk-major transposed tiles (64 chunks)
    wc1p = np.zeros((C1, NP_ * H2), f32)
    wc1p[:, :N * H2] = wc1
    wc1_sb = np.ascontiguousarray(
        wc1p.T.reshape(64, 128, C1).transpose(1, 0, 2).reshape(128, 64 * C1)
    ).astype(NPBF16)

    # f32 smalls: b2 (64) | bc1 (2) | bc2 (1) | bc3 (3)
    bc3p = np.zeros(384, f32); bc3p[:OUT] = bc3
    sf = np.concatenate([
        b2_sb, bc1.reshape(2, 128).T, bc2.reshape(C2, 1),
        bc3p.reshape(3, 128).T], axis=1).astype(f32)
    sf = np.ascontiguousarray(sf)

    # bf16 smalls: wc2 (256) | wc3 (300)
    wc2_sb = wc2.T.reshape(2, 128, C2).transpose(1, 0, 2).reshape(128, 256)
    sb = np.ascontiguousarray(
        np.concatenate([wc2_sb, wc3.T], axis=1)).astype(NPBF16)

    shared = {
        "w1": w1_sb, "w2": w2_sb, "wc1": wc1_sb, "sf": sf, "sb": sb,
    }
    in_maps = []
    for c in range(NCORES):
        m = dict(shared)
        m["xq"] = np.ascontiguousarray(xq[:, BC * c:BC * (c + 1)])
        in_maps.append(m)
    return in_maps


def run(in_maps, trace=False):
    nc = _build_program()
    return run_bass_kernel_spmd(nc, in_maps, list(range(NCORES)), trace=trace)


def unpack(res):
    y = np.empty((B, OUT), np.float32)
    for c in range(NCORES):
        o = res.results[c]["out"]
        for i, m in ((0, 128), (1, 128), (2, 44)):
            y[BC * c:BC * (c + 1), 128 * i:128 * i + m] = \
                o[0:m, BC * i:BC * (i + 1)].T
    return y


def kernel(x, bw1, bb1, bw2, bb2, wc1, bc1, wc2, bc2, wc3, bc3):
    args = [np.asarray(a, np.float32) for a in
            (x, bw1, bb1, bw2, bb2, wc1, bc1, wc2, bc2, wc3, bc3)]
    in_maps = preprocess(*args)
    res = run(in_maps, trace=False)
    return unpack(res)


# revision 30
# speedup vs baseline: 1.0115x; 1.0115x over previous
"""Trainium2 Bass kernel for nn_KAN_63230508532179 (dense_mlp).

Model (per reference):
  h = gelu(x[:,:,None] * bw1 + bb1)            # [B,1000,16]
  f = tanh(einsum('bnh,noh->bno', h, bw2)+bb2) # [B,1000,8]
  z = f.reshape(B, 8000)
  z = gelu(z @ wc1.T + bc1)                    # [B,256]
  z = gelu(z @ wc2.T + bc2)                    # [B,128]
  y = z @ wc3.T + bc3                          # [B,300]

Strategy: data-parallel over batch across 8 cores (512 rows each); all
on-chip tensors transposed ([feature, batch]).  Branches run in
8-branch groups, two groups per combiner-1 K-chunk.  Layer 1 is K=27
row-tiled matmuls: x ships in supertiles of 12 groups (4 strips x 3
group-slots of 9 rows: 8 x-rows + a ones row, so scale and bias fold
into the weights) and each group's lhsT is a [27,128] block that is
zero outside its 9 rows.  Layer 2 is K=128 M=64 matmuls with 2-way
column concurrency; its bias rides the tanh activation's per-partition
bias port.  Combiner-1 accumulates 63 K-chunks into persistent PSUM.
Every constant is split into per-chunk tiles so Tile's dependency
tracking lets compute start as soon as the first slices land, and the
ACT stream is software-pipelined (tanh of chunk c-1 issues after gelu
of chunk c) so the scalar engine never waits on layer-2 latency.
"""

import os
import sys
from contextlib import ExitStack

sys.path.insert(0, "/opt/trn_rl_repo")
os.environ.setdefault("MYCRO_LOCAL_CACHE", "1")

import numpy as np
import ml_dtypes

import concourse.bass as bass
import concourse.tile as tile
from concourse import bacc, mybir
from concourse.bass_utils import run_bass_kernel_spmd

BF16 = mybir.dt.bfloat16
F32 = mybir.dt.float32
NPBF16 = ml_dtypes.bfloat16

B, N, H1, H2 = 4096, 1000, 16, 8
C1, C2, OUT = 256, 128, 300
NCORES = 8
BC = B // NCORES          # 512 batch rows per core
NCH = 63                  # combiner-1 K-chunks (2 groups each)
NG = 128                  # padded groups of 8 branches (125 real)
NP_ = NG * 8              # 1024 padded branches
NST = 11                  # x supertiles of 12 groups (132 slots)

_CACHE = {}


def _gloc(g):
    """group -> (supertile, strip, slot)"""
    return g // 12, (g % 12) // 3, g % 3


def _build_program():
    if "nc" in _CACHE:
        return _CACHE["nc"]

    nc = bacc.Bacc("TRN2", target_bir_lowering=False, debug=False,
                   num_devices=NCORES)

    xq_d = nc.dram_tensor("xq", [NST * 128, BC], BF16, kind="ExternalInput")
    w1_d = nc.dram_tensor("w1", [NST * 128, 384], BF16, kind="ExternalInput")
    w2_d = nc.dram_tensor("w2", [128, NG * 64], BF16, kind="ExternalInput")
    # f32 smalls packed: b2 (64) | bc1 (2) | bc2 (1) | bc3 (3)
    sf_d = nc.dram_tensor("sf", [128, 70], F32, kind="ExternalInput")
    wc1_d = nc.dram_tensor("wc1", [128, 64 * 256], BF16, kind="ExternalInput")
    # bf16 smalls packed: wc2 (256) | wc3 (300)
    sb_d = nc.dram_tensor("sb", [128, 556], BF16, kind="ExternalInput")
    out_d = nc.dram_tensor("out", [128, 3 * BC], F32, kind="ExternalOutput")

    AF = mybir.ActivationFunctionType

    with ExitStack() as ctx:
        tc = ctx.enter_context(tile.TileContext(nc))
        consts = ctx.enter_context(tc.tile_pool(name="consts", bufs=1))
        h_pool = ctx.enter_context(tc.tile_pool(name="h", bufs=4))
        f_pool = ctx.enter_context(tc.tile_pool(name="f", bufs=4))
        z_pool = ctx.enter_context(tc.tile_pool(name="z", bufs=1))
        ps_h = ctx.enter_context(tc.tile_pool(name="psh", bufs=2, space="PSUM"))
        ps_f = ctx.enter_context(tc.tile_pool(name="psf", bufs=2, space="PSUM"))
        ps_z = ctx.enter_context(tc.tile_pool(name="psz", bufs=1, space="PSUM"))

        # ---- PE warm-up tile: matmuls on it accumulate into z1a, which the
        # first real combiner-1 matmul (start=True) clears, so the values
        # never matter -- they only give the HAM clock gate the gap-free
        # busy window it needs to reach 8/8 (per-chunk sem micro-gaps never
        # do).  Left uninitialized so the burst needs no memset and starts
        # the moment the engines boot.
        warm_sb = consts.tile([128, BC], BF16, tag="warm")

        # ---- chunked constants: one tile per slice => fine-grained deps.
        # Sync issues DMA descriptors serially (~0.6us each), so order by
        # first use: chunk 0's tiles, then f32 smalls, then the rest.
        xe, w1t, w2t, wc1t = [], [], [], []
        sfs = consts.tile([128, 70], F32, tag="sf")
        sbs = consts.tile([128, 556], BF16, tag="sbc")
        for v in range(NST):
            xt = consts.tile([128, BC], BF16, tag=f"xe{v}")
            nc.sync.dma_start(out=xt[:], in_=xq_d[128 * v:128 * (v + 1), :])
            xe.append(xt)
            wt = consts.tile([128, 384], BF16, tag=f"w1_{v}")
            nc.sync.dma_start(out=wt[:], in_=w1_d[128 * v:128 * (v + 1), :])
            w1t.append(wt)
            if v < 8:
                w2c = consts.tile([128, 1024], BF16, tag=f"w2_{v}")
                nc.sync.dma_start(out=w2c[:],
                                  in_=w2_d[:, 1024 * v:1024 * (v + 1)])
                w2t.append(w2c)
                wcc = consts.tile([128, 2048], BF16, tag=f"wc1_{v}")
                nc.sync.dma_start(out=wcc[:],
                                  in_=wc1_d[:, 2048 * v:2048 * (v + 1)])
                wc1t.append(wcc)
            if v == 0:
                nc.sync.dma_start(out=sfs[:], in_=sf_d[:, :])
            elif v == 1:
                nc.sync.dma_start(out=sbs[:], in_=sb_d[:, :])
        b2s = sfs[:, 0:64]
        bc1s = sfs[:, 64:66]
        bc2s = sfs[:, 66:67]
        bc3s = sfs[:, 67:70]
        wc2s = sbs[:, 0:256]
        wc3s = sbs[:, 256:556]

        def w1_ap_g(g):
            t, i, u = _gloc(g)
            return w1t[t][32 * i:32 * i + 27, 128 * u:128 * (u + 1)]

        def x_ap_g(g):
            t, i, u = _gloc(g)
            return xe[t][32 * i:32 * i + 27, :]

        def w2_ap_g(g):
            return w2t[g // 16][:, 64 * (g % 16):64 * (g % 16 + 1)]

        def wc1_ap(c, half):
            return wc1t[c // 8][:, 256 * (c % 8) + 128 * half:
                                256 * (c % 8) + 128 * (half + 1)]

        # ---- main loop: 63 chunks, ACT software-pipelined by one chunk ----
        z1a_ps = ps_z.tile([128, BC], F32, tag="z1a")
        z1b_ps = ps_z.tile([128, BC], F32, tag="z1b")
        fps_q = [None, None]

        def warm_burst(n, start):
            for k in range(n):
                nc.tensor.matmul(z1a_ps[:], lhsT=warm_sb[:, 0:128],
                                 rhs=warm_sb[:], start=(start and k == 0),
                                 stop=False, skip_group_check=True)

        warm_burst(10, True)   # garbage data; z1a cleared by chunk 0's C1
        # zero warm_sb so the mid-loop re-warm bursts (which land after the
        # real accumulation begins) add exact zeros to z1a
        nc.vector.memset(warm_sb[:], 0.0)

        def emit_tanh(c):
            fsb = f_pool.tile([128, BC], BF16, tag="fsb")
            nc.scalar.activation(fsb[:], fps_q[c % 2][:], AF.Tanh,
                                 bias=b2s[:, c:c + 1], scale=1.0)
            return fsb

        def emit_c1(c, fsb):
            last = c == NCH - 1
            nc.tensor.matmul(z1a_ps[:], lhsT=wc1_ap(c, 0), rhs=fsb[:],
                             start=(c == 0), stop=last, skip_group_check=True)
            nc.tensor.matmul(z1b_ps[:], lhsT=wc1_ap(c, 1), rhs=fsb[:],
                             start=(c == 0), stop=last, skip_group_check=True)

        for c in range(NCH):
            hps = ps_h.tile([128, 2 * BC], F32, tag="hps")
            for j in range(2):
                g = 2 * c + j
                i = (g % 12) // 3
                nc.tensor.matmul(
                    hps[:, BC * j:BC * (j + 1)],
                    lhsT=w1_ap_g(g), rhs=x_ap_g(g),
                    start=True, stop=True, tile_position=(32 * i, 0))
            hsb = h_pool.tile([128, 2 * BC], BF16, tag="hsb")
            nc.scalar.activation(hsb[:], hps[:], AF.Gelu)
            if c >= 4:
                fsb = emit_tanh(c - 1)     # pipelined phase
            fps = ps_f.tile([128, BC], F32, tag="fps")
            fps_q[c % 2] = fps
            for j in range(2):
                g = 2 * c + j
                nc.tensor.matmul(
                    fps[64 * j:64 * (j + 1), :],
                    lhsT=w2_ap_g(g),
                    rhs=hsb[:, BC * j:BC * (j + 1)],
                    start=True, stop=True, tile_position=(0, 64 * j))
            if c <= 2:
                fsb = emit_tanh(c)         # fill phase: no pipeline bubble
            if c <= 2 or c >= 4:
                emit_c1(c if c <= 2 else c - 1, fsb)
            if c in (1, 3, 5):
                # keep the HAM busy-window alive through pipeline fill
                warm_burst(8, False)
        fsb = emit_tanh(NCH - 1)
        emit_c1(NCH - 1, fsb)

        # ---- combiner tail ----
        z1a = z_pool.tile([128, BC], BF16, tag="z1a_sb")
        z1b = z_pool.tile([128, BC], BF16, tag="z1b_sb")
        z2_ps = ps_h.tile([128, BC], F32, tag="hps")
        nc.scalar.activation(z1a[:], z1a_ps[:], AF.Gelu,
                             bias=bc1s[:, 0:1], scale=1.0)
        nc.tensor.matmul(z2_ps[:], lhsT=wc2s[:, 0:128], rhs=z1a[:],
                         start=True, stop=False, skip_group_check=True)
        nc.scalar.activation(z1b[:], z1b_ps[:], AF.Gelu,
                             bias=bc1s[:, 1:2], scale=1.0)
        nc.tensor.matmul(z2_ps[:], lhsT=wc2s[:, 128:256], rhs=z1b[:],
                         start=False, stop=True, skip_group_check=True)
        z2 = z_pool.tile([128, BC], BF16, tag="z2_sb")
        nc.scalar.activation(z2[:], z2_ps[:], AF.Gelu,
                             bias=bc2s[:, 0:1], scale=1.0)

        o_sb = z_pool.tile([128, 3 * BC], F32, tag="osb")
        for i, m in ((0, 128), (1, 128), (2, 44)):
            o_ps = ps_f.tile([128, BC], F32, tag="fps")
            nc.tensor.matmul(o_ps[0:m, :], lhsT=wc3s[:, 128 * i:128 * i + m],
                             rhs=z2[:], start=True, stop=True)
            nc.vector.tensor_scalar_add(o_sb[0:m, BC * i:BC * (i + 1)],
                                        o_ps[0:m, :], bc3s[0:m, i:i + 1])
            if i == 1:
                nc.sync.dma_start(out=out_d[:, 0:2 * BC],
                                  in_=o_sb[:, 0:2 * BC])
        nc.sync.dma_start(out=out_d[:, 2 * BC:], in_=o_sb[:, 2 * BC:])

    nc.compile()
    _CACHE["nc"] = nc
    return nc


def preprocess(x, bw1, bb1, bw2, bb2, wc1, bc1, wc2, bc2, wc3, bc3):
    """Host-side repack of full inputs into per-core input maps."""
    f32 = np.float32
    bw1p = np.zeros((NST * 12 * 8, H1), f32); bw1p[:N] = bw1
    bb1p = np.zeros((NST * 12 * 8, H1), f32); bb1p[:N] = bb1
    bw2p = np.zeros((NP_, H2, H1), f32); bw2p[:N] = bw2
    bb2p = np.zeros((NP_, H2), f32); bb2p[:N] = bb2

    # x supertiles: [t, 128, B]; strip i, slot u, row r (8 x rows + ones)
    xr = np.zeros((NST * 12 * 8, B), f32)
    xr[:N] = x.T
    xrg = xr.reshape(NST, 4, 3, 8, B)          # [t, i, u, r, b]
    xq = np.zeros((NST, 4, 32, B), f32)
    for u in range(3):
        xq[:, :, 9 * u:9 * u + 8, :] = xrg[:, :, u]
        xq[:, :, 9 * u + 8, :] = 1.0
    xq = xq.reshape(NST * 128, B).astype(NPBF16)

    # w1 blocks: [t, i (32-row strip), z=9u+r, 128u + (16 bb + k)]
    W1 = np.zeros((NST, 4, 32, 384), f32)
    bw1g = bw1p.reshape(NST, 4, 3, 8, H1)      # [t, i, u, bb, k]
    bb1g = bb1p.reshape(NST, 4, 3, 8, H1)
    for u in range(3):
        for bb in range(8):
            W1[:, :, 9 * u + bb, 128 * u + 16 * bb:128 * u + 16 * bb + 16] = \
                bw1g[:, :, u, bb]
            W1[:, :, 9 * u + 8, 128 * u + 16 * bb:128 * u + 16 * bb + 16] = \
                bb1g[:, :, u, bb]
    w1_sb = W1.reshape(NST * 128, 384).astype(NPBF16)

    # w2 block-diagonal per group: [128=(bb,k), 64=(bb,o)]
    W2 = np.zeros((NG, 128, 64), f32)
    bw2g = bw2p.reshape(NG, 8, H2, H1)         # [g, bb, o, k]
    for bb in range(8):
        W2[:, 16 * bb:16 * (bb + 1), 8 * bb:8 * (bb + 1)] = \
            bw2g[:, bb].transpose(0, 2, 1)     # [g, k, o]
    w2_sb = W2.transpose(1, 0, 2).reshape(128, NG * 64).astype(NPBF16)
    b2_sb = bb2p.reshape(64, 128).T

    # combiner 1: wc1 [256, 8000] -> chunk-major transposed tiles (64 chunks)
    wc1p = np.zeros((C1, NP_ * H2), f32)
    wc1p[:, :N * H2] = wc1
    wc1_sb = np.ascontiguousarray(
        wc1p.T.reshape(64, 128, C1).transpose(1, 0, 2).reshape(128, 64 * C1)
    ).astype(NPBF16)

    # f32 smalls: b2 (64) | bc1 (2) | bc2 (1) | bc3 (3)
    bc3p = np.zeros(384, f32); bc3p[:OUT] = bc3
    sf = np.concatenate([
        b2_sb, bc1.reshape(2, 128).T, bc2.reshape(C2, 1),
        bc3p.reshape(3, 128).T], axis=1).astype(f32)
    sf = np.ascontiguousarray(sf)

    # bf16 smalls: wc2 (256) | wc3 (300)
    wc2_sb = wc2.T.reshape(2, 128, C2).transpose(1, 0, 2).reshape(128, 256)
    sb = np.ascontiguousarray(
        np.concatenate([wc2_sb, wc3.T], axis=1)).astype(NPBF16)

    shared = {
        "w1": w1_sb, "w2": w2_sb, "wc1": wc1_sb, "sf": sf, "sb": sb,
    }
    in_maps = []
    for c in range(NCORES):
        m = dict(shared)
        m["xq"] = np.ascontiguousarray(xq[:, BC * c:BC * (c + 1)])
        in_maps.append(m)
    return in_maps


def run(in_maps, trace=False):
    nc = _build_program()
    return run_bass_kernel_spmd(nc, in_maps, list(range(NCORES)), trace=trace)


def unpack(res):
    y = np.empty((B, OUT), np.float32)
    for c in range(NCORES):
        o = res.results[c]["out"]
        for i, m in ((0, 128), (1, 128), (2, 44)):
            y[BC * c:BC * (c + 1), 128 * i:128 * i + m] = \
                o[0:m, BC * i:BC * (i + 1)].T
    return y


def kernel(x, bw1, bb1, bw2, bb2, wc1, bc1, wc2, bc2, wc3, bc3):
    args = [np.asarray(a, np.float32) for a in
            (x, bw1, bb1, bw2, bb2, wc1, bc1, wc2, bc2, wc3, bc3)]
    in_maps = preprocess(*args)
    res = run(in_maps, trace=False)
    return unpack(res)


# revision 33
# speedup vs baseline: 1.0274x; 1.0157x over previous
"""Trainium2 Bass kernel for nn_KAN_63230508532179 (dense_mlp).

Model (per reference):
  h = gelu(x[:,:,None] * bw1 + bb1)            # [B,1000,16]
  f = tanh(einsum('bnh,noh->bno', h, bw2)+bb2) # [B,1000,8]
  z = f.reshape(B, 8000)
  z = gelu(z @ wc1.T + bc1)                    # [B,256]
  z = gelu(z @ wc2.T + bc2)                    # [B,128]
  y = z @ wc3.T + bc3                          # [B,300]

Strategy: data-parallel over batch across 8 cores (512 rows each); all
on-chip tensors transposed ([feature, batch]).  Branches run in
8-branch groups, two groups per combiner-1 K-chunk.  Layer 1 is K=27
row-tiled matmuls: x ships in supertiles of 12 groups (4 strips x 3
group-slots of 9 rows: 8 x-rows + a ones row, so scale and bias fold
into the weights) and each group's lhsT is a [27,128] block that is
zero outside its 9 rows.  Layer 2 is K=128 M=64 matmuls with 2-way
column concurrency; its bias rides the tanh activation's per-partition
bias port.  Combiner-1 accumulates 63 K-chunks into persistent PSUM.
Every constant is split into per-chunk tiles so Tile's dependency
tracking lets compute start as soon as the first slices land, and the
ACT stream is software-pipelined (tanh of chunk c-1 issues after gelu
of chunk c) so the scalar engine never waits on layer-2 latency.
"""

import os
import sys
from contextlib import ExitStack

sys.path.insert(0, "/opt/trn_rl_repo")
os.environ.setdefault("MYCRO_LOCAL_CACHE", "1")

import numpy as np
import ml_dtypes

import concourse.bass as bass
import concourse.tile as tile
from concourse import bacc, mybir
from concourse.bass_utils import run_bass_kernel_spmd

BF16 = mybir.dt.bfloat16
F32 = mybir.dt.float32
NPBF16 = ml_dtypes.bfloat16

B, N, H1, H2 = 4096, 1000, 16, 8
C1, C2, OUT = 256, 128, 300
NCORES = 8
BC = B // NCORES          # 512 batch rows per core
NCH = 63                  # combiner-1 K-chunks (2 groups each)
NG = 128                  # padded groups of 8 branches (125 real)
NP_ = NG * 8              # 1024 padded branches
NST = 11                  # x supertiles of 12 groups (132 slots)

_CACHE = {}


def _gloc(g):
    """group -> (supertile, strip, slot)"""
    return g // 12, (g % 12) // 3, g % 3


def _build_program():
    if "nc" in _CACHE:
        return _CACHE["nc"]

    nc = bacc.Bacc("TRN2", target_bir_lowering=False, debug=False,
                   num_devices=NCORES)

    xq_d = nc.dram_tensor("xq", [NST * 128, BC], BF16, kind="ExternalInput")
    w1_d = nc.dram_tensor("w1", [NST * 128, 384], BF16, kind="ExternalInput")
    w2_d = nc.dram_tensor("w2", [128, NG * 64], BF16, kind="ExternalInput")
    # f32 smalls packed: b2 (64) | bc1 (2) | bc2 (1) | bc3 (3)
    sf_d = nc.dram_tensor("sf", [128, 70], F32, kind="ExternalInput")
    wc1_d = nc.dram_tensor("wc1", [128, 64 * 256], BF16, kind="ExternalInput")
    # bf16 smalls packed: wc2 (256) | wc3 (300)
    sb_d = nc.dram_tensor("sb", [128, 556], BF16, kind="ExternalInput")
    out_d = nc.dram_tensor("out", [128, 3 * BC], F32, kind="ExternalOutput")

    AF = mybir.ActivationFunctionType

    with ExitStack() as ctx:
        tc = ctx.enter_context(tile.TileContext(nc))
        consts = ctx.enter_context(tc.tile_pool(name="consts", bufs=1))
        h_pool = ctx.enter_context(tc.tile_pool(name="h", bufs=4))
        f_pool = ctx.enter_context(tc.tile_pool(name="f", bufs=4))
        z_pool = ctx.enter_context(tc.tile_pool(name="z", bufs=1))
        ps_h = ctx.enter_context(tc.tile_pool(name="psh", bufs=2, space="PSUM"))
        ps_f = ctx.enter_context(tc.tile_pool(name="psf", bufs=2, space="PSUM"))
        ps_z = ctx.enter_context(tc.tile_pool(name="psz", bufs=1, space="PSUM"))

        # ---- PE warm-up tile: matmuls on it accumulate into z1a, which the
        # first real combiner-1 matmul (start=True) clears, so the values
        # never matter -- they only give the HAM clock gate the gap-free
        # busy window it needs to reach 8/8 (per-chunk sem micro-gaps never
        # do).  Left uninitialized so the burst needs no memset and starts
        # the moment the engines boot.
        warm_sb = consts.tile([128, BC], BF16, tag="warm")

        # ---- chunked constants: one tile per slice => fine-grained deps.
        # Sync issues DMA descriptors serially (~0.6us each), so order by
        # first use: chunk 0's tiles, then f32 smalls, then the rest.
        xe, w1t, w2t, wc1t = [], [], [], []
        sfs = consts.tile([128, 70], F32, tag="sf")
        sbs = consts.tile([128, 556], BF16, tag="sbc")
        for v in range(NST):
            xt = consts.tile([128, BC], BF16, tag=f"xe{v}")
            nc.sync.dma_start(out=xt[:], in_=xq_d[128 * v:128 * (v + 1), :])
            xe.append(xt)
            wt = consts.tile([128, 384], BF16, tag=f"w1_{v}")
            nc.sync.dma_start(out=wt[:], in_=w1_d[128 * v:128 * (v + 1), :])
            w1t.append(wt)
            if v < 8:
                w2c = consts.tile([128, 1024], BF16, tag=f"w2_{v}")
                nc.sync.dma_start(out=w2c[:],
                                  in_=w2_d[:, 1024 * v:1024 * (v + 1)])
                w2t.append(w2c)
                wcc = consts.tile([128, 2048], BF16, tag=f"wc1_{v}")
                nc.sync.dma_start(out=wcc[:],
                                  in_=wc1_d[:, 2048 * v:2048 * (v + 1)])
                wc1t.append(wcc)
            if v == 0:
                nc.sync.dma_start(out=sfs[:], in_=sf_d[:, :])
            elif v == 1:
                nc.sync.dma_start(out=sbs[:], in_=sb_d[:, :])
        b2s = sfs[:, 0:64]
        bc1s = sfs[:, 64:66]
        bc2s = sfs[:, 66:67]
        bc3s = sfs[:, 67:70]
        wc2s = sbs[:, 0:256]
        wc3s = sbs[:, 256:556]

        def w1_ap_g(g):
            t, i, u = _gloc(g)
            return w1t[t][32 * i:32 * i + 27, 128 * u:128 * (u + 1)]

        def x_ap_g(g):
            t, i, u = _gloc(g)
            return xe[t][32 * i:32 * i + 27, :]

        def w2_ap_g(g):
            return w2t[g // 16][:, 64 * (g % 16):64 * (g % 16 + 1)]

        def wc1_ap(c, half):
            return wc1t[c // 8][:, 256 * (c % 8) + 128 * half:
                                256 * (c % 8) + 128 * (half + 1)]

        # ---- main loop: 63 chunks, ACT software-pipelined by one chunk ----
        z1a_ps = ps_z.tile([128, BC], F32, tag="z1a")
        z1b_ps = ps_z.tile([128, BC], F32, tag="z1b")
        fps_q = [None, None]

        def warm_burst(n, start):
            for k in range(n):
                nc.tensor.matmul(z1a_ps[:], lhsT=warm_sb[:, 0:128],
                                 rhs=warm_sb[:], start=(start and k == 0),
                                 stop=False, skip_group_check=True)

        warm_burst(8, True)    # garbage data; z1a cleared by chunk 0's C1
        # zero warm_sb so the mid-loop re-warm bursts (which land after the
        # real accumulation begins) add exact zeros to z1a
        nc.vector.memset(warm_sb[:], 0.0)

        def emit_tanh(c):
            fsb = f_pool.tile([128, BC], BF16, tag="fsb")
            nc.scalar.activation(fsb[:], fps_q[c % 2][:], AF.Tanh,
                                 bias=b2s[:, c:c + 1], scale=1.0)
            return fsb

        def emit_c1(c, fsb):
            last = c == NCH - 1
            nc.tensor.matmul(z1a_ps[:], lhsT=wc1_ap(c, 0), rhs=fsb[:],
                             start=(c == 0), stop=last, skip_group_check=True)
            nc.tensor.matmul(z1b_ps[:], lhsT=wc1_ap(c, 1), rhs=fsb[:],
                             start=(c == 0), stop=last, skip_group_check=True)

        for c in range(NCH):
            hps = ps_h.tile([128, 2 * BC], F32, tag="hps")
            for j in range(2):
                g = 2 * c + j
                i = (g % 12) // 3
                nc.tensor.matmul(
                    hps[:, BC * j:BC * (j + 1)],
                    lhsT=w1_ap_g(g), rhs=x_ap_g(g),
                    start=True, stop=True, tile_position=(32 * i, 0))
            hsb = h_pool.tile([128, 2 * BC], BF16, tag="hsb")
            nc.scalar.activation(hsb[:], hps[:], AF.Gelu)
            if c < 8:
                # micro-burst in the PE's gelu-wait slot: keeps the HAM
                # busy-window alive through fill without delaying L2/C1
                warm_burst(4, False)
            if c >= 4:
                fsb = emit_tanh(c - 1)     # pipelined phase
            fps = ps_f.tile([128, BC], F32, tag="fps")
            fps_q[c % 2] = fps
            for j in range(2):
                g = 2 * c + j
                nc.tensor.matmul(
                    fps[64 * j:64 * (j + 1), :],
                    lhsT=w2_ap_g(g),
                    rhs=hsb[:, BC * j:BC * (j + 1)],
                    start=True, stop=True, tile_position=(0, 64 * j))
            if c <= 2:
                fsb = emit_tanh(c)         # fill phase: no pipeline bubble
            if c <= 2 or c >= 4:
                emit_c1(c if c <= 2 else c - 1, fsb)
        fsb = emit_tanh(NCH - 1)
        emit_c1(NCH - 1, fsb)

        # ---- combiner tail ----
        z1a = z_pool.tile([128, BC], BF16, tag="z1a_sb")
        z1b = z_pool.tile([128, BC], BF16, tag="z1b_sb")
        z2_ps = ps_h.tile([128, BC], F32, tag="hps")
        nc.scalar.activation(z1a[:], z1a_ps[:], AF.Gelu,
                             bias=bc1s[:, 0:1], scale=1.0)
        nc.tensor.matmul(z2_ps[:], lhsT=wc2s[:, 0:128], rhs=z1a[:],
                         start=True, stop=False, skip_group_check=True)
        nc.scalar.activation(z1b[:], z1b_ps[:], AF.Gelu,
                             bias=bc1s[:, 1:2], scale=1.0)
        nc.tensor.matmul(z2_ps[:], lhsT=wc2s[:, 128:256], rhs=z1b[:],
                         start=False, stop=True, skip_group_check=True)
        z2 = z_pool.tile([128, BC], BF16, tag="z2_sb")
        nc.scalar.activation(z2[:], z2_ps[:], AF.Gelu,
                             bias=bc2s[:, 0:1], scale=1.0)

        o_sb = z_pool.tile([128, 3 * BC], F32, tag="osb")
        for i, m in ((0, 128), (1, 128), (2, 44)):
            o_ps = ps_f.tile([128, BC], F32, tag="fps")
            nc.tensor.matmul(o_ps[0:m, :], lhsT=wc3s[:, 128 * i:128 * i + m],
                             rhs=z2[:], start=True, stop=True)
            nc.vector.tensor_scalar_add(o_sb[0:m, BC * i:BC * (i + 1)],
                                        o_ps[0:m, :], bc3s[0:m, i:i + 1])
            if i == 1:
                nc.sync.dma_start(out=out_d[:, 0:2 * BC],
                                  in_=o_sb[:, 0:2 * BC])
        nc.sync.dma_start(out=out_d[:, 2 * BC:], in_=o_sb[:, 2 * BC:])

    nc.compile()
    _CACHE["nc"] = nc
    return nc


def preprocess(x, bw1, bb1, bw2, bb2, wc1, bc1, wc2, bc2, wc3, bc3):
    """Host-side repack of full inputs into per-core input maps."""
    f32 = np.float32
    bw1p = np.zeros((NST * 12 * 8, H1), f32); bw1p[:N] = bw1
    bb1p = np.zeros((NST * 12 * 8, H1), f32); bb1p[:N] = bb1
    bw2p = np.zeros((NP_, H2, H1), f32); bw2p[:N] = bw2
    bb2p = np.zeros((NP_, H2), f32); bb2p[:N] = bb2

    # x supertiles: [t, 128, B]; strip i, slot u, row r (8 x rows + ones)
    xr = np.zeros((NST * 12 * 8, B), f32)
    xr[:N] = x.T
    xrg = xr.reshape(NST, 4, 3, 8, B)          # [t, i, u, r, b]
    xq = np.zeros((NST, 4, 32, B), f32)
    for u in range(3):
        xq[:, :, 9 * u:9 * u + 8, :] = xrg[:, :, u]
        xq[:, :, 9 * u + 8, :] = 1.0
    xq = xq.reshape(NST * 128, B).astype(NPBF16)

    # w1 blocks: [t, i (32-row strip), z=9u+r, 128u + (16 bb + k)]
    W1 = np.zeros((NST, 4, 32, 384), f32)
    bw1g = bw1p.reshape(NST, 4, 3, 8, H1)      # [t, i, u, bb, k]
    bb1g = bb1p.reshape(NST, 4, 3, 8, H1)
    for u in range(3):
        for bb in range(8):
            W1[:, :, 9 * u + bb, 128 * u + 16 * bb:128 * u + 16 * bb + 16] = \
                bw1g[:, :, u, bb]
            W1[:, :, 9 * u + 8, 128 * u + 16 * bb:128 * u + 16 * bb + 16] = \
                bb1g[:, :, u, bb]
    w1_sb = W1.reshape(NST * 128, 384).astype(NPBF16)

    # w2 block-diagonal per group: [128=(bb,k), 64=(bb,o)]
    W2 = np.zeros((NG, 128, 64), f32)
    bw2g = bw2p.reshape(NG, 8, H2, H1)         # [g, bb, o, k]
    for bb in range(8):
        W2[:, 16 * bb:16 * (bb + 1), 8 * bb:8 * (bb + 1)] = \
            bw2g[:, bb].transpose(0, 2, 1)     # [g, k, o]
    w2_sb = W2.transpose(1, 0, 2).reshape(128, NG * 64).astype(NPBF16)
    b2_sb = bb2p.reshape(64, 128).T

    # combiner 1: wc1 [256, 8000] -> chunk-major transposed tiles (64 chunks)
    wc1p = np.zeros((C1, NP_ * H2), f32)
    wc1p[:, :N * H2] = wc1
    wc1_sb = np.ascontiguousarray(
        wc1p.T.reshape(64, 128, C1).transpose(1, 0, 2).reshape(128, 64 * C1)
    ).astype(NPBF16)

    # f32 smalls: b2 (64) | bc1 (2) | bc2 (1) | bc3 (3)
    bc3p = np.zeros(384, f32); bc3p[:OUT] = bc3
    sf = np.concatenate([
        b2_sb, bc1.reshape(2, 128).T, bc2.reshape(C2, 1),
        bc3p.reshape(3, 128).T], axis=1).astype(f32)
    sf = np.ascontiguousarray(sf)

    # bf16 smalls: wc2 (256) | wc3 (300)
    wc2_sb = wc2.T.reshape(2, 128, C2).transpose(1, 0, 2).reshape(128, 256)
    sb = np.ascontiguousarray(
        np.concatenate([wc2_sb, wc3.T], axis=1)).astype(NPBF16)

    shared = {
        "w1": w1_sb, "w2": w2_sb, "wc1": wc1_sb, "sf": sf, "sb": sb,
    }
    in_maps = []
    for c in range(NCORES):
        m = dict(shared)
        m["xq"] = np.ascontiguousarray(xq[:, BC * c:BC * (c + 1)])
        in_maps.append(m)
    return in_maps


def run(in_maps, trace=False):
    nc = _build_program()
    return run_bass_kernel_spmd(nc, in_maps, list(range(NCORES)), trace=trace)


def unpack(res):
    y = np.empty((B, OUT), np.float32)
    for c in range(NCORES):
        o = res.results[c]["out"]
        for i, m in ((0, 128), (1, 128), (2, 44)):
            y[BC * c:BC * (c + 1), 128 * i:128 * i + m] = \
                o[0:m, BC * i:BC * (i + 1)].T
    return y


def kernel(x, bw1, bb1, bw2, bb2, wc1, bc1, wc2, bc2, wc3, bc3):
    args = [np.asarray(a, np.float32) for a in
            (x, bw1, bb1, bw2, bb2, wc1, bc1, wc2, bc2, wc3, bc3)]
    in_maps = preprocess(*args)
    res = run(in_maps, trace=False)
    return unpack(res)
